# revision 1
# baseline (speedup 1.0000x reference)
"""Trainium2 Bass kernel for nn_MultiHeadAttention_27693949124623.

Math (validated vs reference in fp64 numpy):
  The reference's "faithful split_heads" reinterprets the [H=8, B*N=512, 16]
  per-head projections as 16 independent full-attention instances
  (h2 in 0..8) x (b in 0..2), each over an effective sequence of 2048
  positions with head dim 2 and softmax over all 2048 keys:
     q index s in [0,2048):  g = 4*b + s//512 (original projection head),
                             t = s % 512     (row of qflat),
     per-instance Q/K/V components = column 2*h2+e of head g's projection.
  Output row r = b*256+n, col c = 16*j + 2*h2 + e gathers head outputs at
  s = 8*n+j, multiplied by W_out.reshape(128,128).

Sharding: core c handles h2 = c (head-parallel), both b instances.
Each core returns a partial [512, 128] (its h2 slice of W_out rows applied);
the host sums the 8 partials (the unshard step) and reshapes to (2,256,128).

Device pipeline per core (all matmuls via float32r = full-rate fp32 stream):
  - transpose q -> qT [d, t]
  - project Q^T / K^T directly into 32-aligned partition "stages" via
    column-tiled matmuls (so score matmuls can row-pack with K=2 lhsT)
  - scores s^T[k-chunk 128, q 1024] = K_chunk^T.T @ Q^T on PE,
    exp on ScalarE (PSUM -> SBUF),
    p^T @ [Vx, Vy, 1] accumulated on PE -> [num_x, num_y, den] per q
  - divide, gather to [16, 512] via strided SBUF DMAs, final [16]x[128]
    matmuls against the core's W_out slice.
"""

import sys

for _p in ("/opt/trn_rl_repo",):
    if _p not in sys.path:
        sys.path.insert(0, _p)

from contextlib import ExitStack

import numpy as np

import concourse.bass as bass
import concourse.tile as tile
from concourse import mybir
from concourse.bass_utils import run_bass_kernel_spmd

F32 = mybir.dt.float32
F32R = mybir.dt.float32r
AF = mybir.ActivationFunctionType

NCORES = 8


def _emit(ctx: ExitStack, tc: tile.TileContext, io: dict):
    nc = tc.nc
    qin = io["qin"].ap()      # [512, 128]
    wq = io["wq"].ap()        # [128, 16] col = 2g+e, pre-scaled by 0.5
    wk = io["wk"].ap()        # [128, 16] col = 2g+e, pre-scaled by 0.5
    wv = io["wv"].ap()        # [128, 16] col = 8e+g
    wsub = io["wsub"].ap()    # [16, 128] row = 2j+e
    ident = io["ident"].ap()  # [128, 128]
    outp = io["outp"].ap()    # [512, 128]

    const = ctx.enter_context(tc.tile_pool(name="const", bufs=1))
    sbp = ctx.enter_context(tc.tile_pool(name="sbp", bufs=3))
    sbi = ctx.enter_context(tc.tile_pool(name="sbi", bufs=2))
    sbm = ctx.enter_context(tc.tile_pool(name="sbm", bufs=1))
    # PSUM budget (8 banks of 2KB/partition):
    #   misc: prep tiles [<=128, 512] x2 bufs        = 2 banks
    #   psc:  scores [128,1024] x2 bufs              = 4 banks
    #   pac:  acc0 + acc1 (also ptp)                 = 2 banks
    # (walrus allows only ONE sync wait per engine instruction; _split_waits
    # post-processes the scheduled program to hoist extras into NoOps)
    misc = ctx.enter_context(tc.tile_pool(name="misc", bufs=2, space="PSUM"))
    psc = ctx.enter_context(tc.tile_pool(name="psc", bufs=2, space="PSUM"))
    pac = ctx.enter_context(tc.tile_pool(name="pac", bufs=1, space="PSUM"))

    # ---- constant loads -------------------------------------------------
    q_sb = const.tile([128, 4, 128], F32, name="q_sb")
    nc.sync.dma_start(out=q_sb, in_=qin.rearrange("(t p) d -> p t d", p=128))
    wq_sb = const.tile([128, 16], F32, name="wq_sb")
    nc.sync.dma_start(out=wq_sb, in_=wq)
    wk_sb = const.tile([128, 16], F32, name="wk_sb")
    nc.sync.dma_start(out=wk_sb, in_=wk)
    wv_sb = const.tile([128, 16], F32, name="wv_sb")
    nc.sync.dma_start(out=wv_sb, in_=wv)
    wsub_sb = const.tile([16, 128], F32, name="wsub_sb")
    nc.sync.dma_start(out=wsub_sb, in_=wsub)
    ident_sb = const.tile([128, 128], F32, name="ident_sb")
    nc.sync.dma_start(out=ident_sb, in_=ident)

    # preload the exp table set early so the ~2.7us load overlaps the DMAs
    dummy = const.tile([1, 8], F32, name="dummy")
    nc.vector.memset(dummy, 0.0)
    nc.scalar.activation(out=dummy, in_=dummy, func=AF.Exp)

    # ---- qT[d, t] via PE transposes (each carries <=1 wait) --------------
    ptp = pac.tile([128, 512], F32, name="ptp", tag="acc1")
    qT = const.tile([128, 4, 128], F32, name="qT")
    for t4 in range(4):
        nc.tensor.transpose(
            ptp[:, 128 * t4 : 128 * (t4 + 1)], q_sb[:, t4, :], ident_sb
        )
    qT_flat = qT.rearrange("p a x -> p (a x)")  # [128, 512]
    nc.vector.tensor_copy(out=qT_flat, in_=ptp)

    # ---- G_v_ext [128, t4, 24]; cols 8e+g (vx, vy); 16..23 = ones --------
    gv = const.tile([128, 4, 24], F32R, name="gv")
    ones_sb = const.tile([128, 8], F32, name="ones_sb")
    nc.vector.memset(ones_sb, 1.0)
    for t4 in range(4):
        nc.vector.tensor_copy(out=gv[:, t4, 16:24], in_=ones_sb)
    for t4 in range(4):
        pg = misc.tile([2, 512], F32, name=f"pg{t4}", tag="misc", padded_shape=[128, 512])
        nc.tensor.matmul(
            pg.tensor.ap()[0:128, 0:16],
            lhsT=qT[:, t4, :],
            rhs=wv_sb,
            start=True,
            stop=True,
        )
        nc.vector.tensor_copy(out=gv[:, t4, 0:16], in_=pg.tensor.ap()[0:128, 0:16])
    gvv = gv.rearrange("p t (e g) -> p t g e", e=3)  # [128, 4, 8, 3]

    MT = sbm.tile([16, 512], F32, name="MT")
    DT = sbm.tile([16, 512], F32, name="DT")

    insts = {}
    for b in (0, 1):
        # ---- project Q^T / K^T into [2, 2048] (gp-major), exact fp32 -----
        # then split into float32r-rounded + residual parts:
        #   GQr + dGQ = Q^T exactly; GKr + dGK = K^T exactly.
        GQr = sbi.tile([2, 2048], F32R, name=f"GQr{b}", tag="GQr")
        dGQ = sbi.tile([2, 2048], F32R, name=f"dGQ{b}", tag="dGQ")
        GKr = sbi.tile([2, 2048], F32R, name=f"GKr{b}", tag="GKr")
        dGK = sbi.tile([2, 2048], F32R, name=f"dGK{b}", tag="dGK")
        for gp in range(4):
            cs = slice(512 * gp, 512 * (gp + 1))
            wcol = slice(8 * b + 2 * gp, 8 * b + 2 * gp + 2)
            pq2 = misc.tile([2, 512], F32, name=f"pq2{b}{gp}", tag="misc")
            nc.tensor.matmul(
                pq2, lhsT=wq_sb[:, wcol], rhs=qT_flat, start=True, stop=True
            )
            nc.vector.tensor_copy(out=GQr[:, cs], in_=pq2)
            nc.vector.tensor_sub(dGQ[:, cs], pq2, GQr[:, cs])
            pk2 = misc.tile([2, 512], F32, name=f"pk2{b}{gp}", tag="misc")
            nc.tensor.matmul(
                pk2, lhsT=wk_sb[:, wcol], rhs=qT_flat, start=True, stop=True
            )
            nc.vector.tensor_copy(out=GKr[:, cs], in_=pk2)
            nc.vector.tensor_sub(dGK[:, cs], pk2, GKr[:, cs])

        # ---- assemble 6-row G6 tensors with DMAs (partition-base-free) ---
        #   GQ6 rows = [ur_x, ur_y, ur_x, ur_y, du_x, du_y]
        #   GK6 rows = [kr_x, kr_y, dk_x, dk_y, kr_x, kr_y]
        # summed over 6 contraction rows: ur*kr + ur*dk + du*kr ~ fp32 score
        GQ6 = sbi.tile([6, 2048], F32R, name=f"GQ6{b}", tag="GQ6")
        GK6 = sbi.tile([6, 2048], F32R, name=f"GK6{b}", tag="GK6")
        nc.sync.dma_start(out=GQ6[0:2, :], in_=GQr)
        nc.sync.dma_start(out=GQ6[2:4, :], in_=GQr)
        nc.sync.dma_start(out=GQ6[4:6, :], in_=dGQ)
        nc.sync.dma_start(out=GK6[0:2, :], in_=GKr)
        nc.sync.dma_start(out=GK6[2:4, :], in_=dGK)
        nc.sync.dma_start(out=GK6[4:6, :], in_=GKr)

        # ---- replicate to the four 32-aligned row groups -----------------
        # Qs[32r+i, q] = GQ6[i, q]  (q = 512*gp + t)
        # Ks2[32t4+i, 512*g2 + t] = GK6[i, 512*g2 + t]
        Qs = sbi.tile([128, 2048], F32R, name=f"Qs{b}", tag="Qs")
        Ks2 = sbi.tile([128, 2048], F32R, name=f"Ks2{b}", tag="Ks2")
        for r in range(4):
            nc.sync.dma_start(out=Qs[32 * r : 32 * r + 6, :], in_=GQ6)
            nc.sync.dma_start(out=Ks2[32 * r : 32 * r + 6, :], in_=GK6)
        insts[b] = (Qs, Ks2)

    for b in (0, 1):
        Qs, Ks2 = insts[b]
        # ---- main attention loop -----------------------------------------
        # Software-skewed by one iteration: the scores matmuls for step i+1
        # are emitted BEFORE the p@V matmuls of step i, so the PE streams
        # score work while ScalarE runs exp(i) instead of stalling behind it.
        accs = sbi.tile([3, 2048], F32, name=f"accs{b}", tag="accs")
        steps = [(h, ci) for h in range(2) for ci in range(16)]

        def emit_scores(i):
            h, ci = steps[i]
            g2, t4 = divmod(ci, 4)
            sps = psc.tile([128, 1024], F32, name=f"sps{b}{h}{ci}", tag="sps")
            for qi in range(2):
                qq = 2 * h + qi
                nc.tensor.matmul(
                    sps[:, 512 * qi : 512 * (qi + 1)],
                    lhsT=Ks2[
                        32 * t4 : 32 * t4 + 6,
                        512 * g2 + 128 * t4 : 512 * g2 + 128 * t4 + 128,
                    ],
                    rhs=Qs[32 * t4 : 32 * t4 + 6, 512 * qq : 512 * (qq + 1)],
                    start=True,
                    stop=True,
                    tile_position=(32 * t4, 0),
                )
            return sps

        accs_cur = {}
        sps_cur = emit_scores(0)
        for i, (h, ci) in enumerate(steps):
            g2, t4 = divmod(ci, 4)
            if ci == 0:
                accs_cur[0] = pac.tile([3, 512], F32, name=f"acc{b}{h}0", tag="acc0")
                accs_cur[1] = pac.tile([3, 512], F32, name=f"acc{b}{h}1", tag="acc1")
            psb = sbp.tile([128, 1024], F32R, name=f"psb{b}{h}{ci}", tag="psb")
            nc.scalar.activation(out=psb, in_=sps_cur, func=AF.Exp)
            sps_nxt = emit_scores(i + 1) if i + 1 < len(steps) else None
            for qi in range(2):
                nc.tensor.matmul(
                    accs_cur[qi][0:3, :],
                    lhsT=gvv[:, t4, 4 * b + g2, :],
                    rhs=psb[:, 512 * qi : 512 * (qi + 1)],
                    start=(ci == 0),
                    stop=(ci == 15),
                )
            sps_cur = sps_nxt
            if ci == 15:
                for qi in range(2):
                    qq = 2 * h + qi
                    nc.vector.tensor_copy(
                        out=accs[:, 512 * qq : 512 * (qq + 1)],
                        in_=accs_cur[qi][0:3, :],
                    )

        # ---- gather to [16, 512]: MT[2j+e, b*256+n] = accs[e, 8n+j] ------
        # spread across issuing engines so the small strided DMAs use
        # multiple DGE queues instead of serializing on SP's
        engs = [nc.sync, nc.sync]
        for j in range(8):
            engs[j % 2].dma_start(
                out=MT[2 * j : 2 * j + 2, 256 * b : 256 * (b + 1)],
                in_=accs[0:2, j::8],
            )
            engs[(j + 1) % 2].dma_start(
                out=DT[2 * j : 2 * j + 1, 256 * b : 256 * (b + 1)],
                in_=accs[2:3, j::8],
            )
            engs[j % 2].dma_start(
                out=DT[2 * j + 1 : 2 * j + 2, 256 * b : 256 * (b + 1)],
                in_=accs[2:3, j::8],
            )

    # ---- divide + final projection (split per instance half so b=0's
    # half runs while b=1's main loop still occupies ScalarE) --------------
    o_sb = sbm.tile([128, 4, 128], F32, name="o_sb")
    for b in (0, 1):
        hs = slice(256 * b, 256 * (b + 1))
        MT2 = sbm.tile([16, 256], F32, name=f"MT2{b}", tag=f"MT2{b}")
        nc.vector.tensor_copy(out=MT2, in_=MT[:, hs])
        DT2 = sbm.tile([16, 256], F32, name=f"DT2{b}", tag=f"DT2{b}")
        nc.vector.tensor_copy(out=DT2, in_=DT[:, hs])
        Dr = sbm.tile([16, 256], F32, name=f"Dr{b}", tag=f"Dr{b}")
        nc.vector.reciprocal(out=Dr, in_=DT2)
        Md = sbm.tile([16, 256], F32, name=f"Md{b}", tag=f"Md{b}")
        nc.vector.tensor_mul(Md, MT2, Dr)
        for ri in range(2):
            rr = 2 * b + ri
            pof = misc.tile([2, 512], F32, name=f"pof{rr}", tag="misc")
            nc.tensor.matmul(
                pof.tensor.ap()[0:128, 0:128],
                lhsT=Md[:, 128 * ri : 128 * (ri + 1)],
                rhs=wsub_sb,
                start=True,
                stop=True,
            )
            nc.vector.tensor_copy(out=o_sb[:, rr, :], in_=pof.tensor.ap()[0:128, 0:128])
            nc.sync.dma_start(out=outp[128 * rr : 128 * (rr + 1), :], in_=o_sb[:, rr, :])


def _split_waits(nc):
    """Walrus codegen allows only ONE sync wait per engine instruction;
    Tile emits as many as the dependencies require. Split extras into
    preceding NoOps on the same engine (NX executes them in order)."""
    import bass_rust

    n = 0
    for fn in nc.m.functions:
        for blk in fn.blocks:
            newl = []
            for ins in blk.instructions:
                si = getattr(ins, "sync_info", None)
                waits = list(si.on_wait) if si is not None and si.on_wait else []
                if len(waits) > 1:
                    for k, w in enumerate(waits[:-1]):
                        nop = mybir.InstNoOp(name=f"{ins.name}-wsp{k}")
                        nop.engine = ins.engine
                        nop.sync_info = bass_rust.SyncInfo(on_wait=[w], on_update=[])
                        newl.append(nop)
                        n += 1
                    ins.sync_info = bass_rust.SyncInfo(
                        on_wait=[waits[-1]], on_update=list(si.on_update)
                    )
                newl.append(ins)
            try:
                blk.instructions = newl
            except Exception:
                blk.instructions.clear()
                blk.instructions.extend(newl)
    return n

def build_program():
    nc = bass.Bass("TRN2", target_bir_lowering=False, debug=False)
    io = {
        "qin": nc.declare_dram_parameter("qin", [512, 128], F32, isOutput=False),
        "wq": nc.declare_dram_parameter("wq", [128, 16], F32, isOutput=False),
        "wk": nc.declare_dram_parameter("wk", [128, 16], F32, isOutput=False),
        "wv": nc.declare_dram_parameter("wv", [128, 16], F32, isOutput=False),
        "wsub": nc.declare_dram_parameter("wsub", [16, 128], F32, isOutput=False),
        "ident": nc.declare_dram_parameter("ident", [128, 128], F32, isOutput=False),
        "outp": nc.declare_dram_parameter("outp", [512, 128], F32, isOutput=True),
    }
    with tile.TileContext(nc) as tc:
        with ExitStack() as ctx:
            _emit(ctx, tc, io)
    _split_waits(nc)
    return nc


def make_in_maps(q, W_query, W_key, W_value, W_out):
    """Shard full inputs into per-core input maps (host-side, tiny)."""
    q = np.asarray(q, np.float32)
    W_query = np.asarray(W_query, np.float32)
    W_key = np.asarray(W_key, np.float32)
    W_value = np.asarray(W_value, np.float32)
    W_out = np.asarray(W_out, np.float32)
    qflat = np.ascontiguousarray(q.reshape(512, 128))
    ident = np.eye(128, dtype=np.float32)
    in_maps = []
    for c in range(NCORES):
        wq = 0.5 * W_query[:, :, 2 * c : 2 * c + 2]  # [8, 128, 2]
        wk = 0.5 * W_key[:, :, 2 * c : 2 * c + 2]
        wv = W_value[:, :, 2 * c : 2 * c + 2]
        in_maps.append(
            {
                "qin": qflat,
                "wq": np.ascontiguousarray(wq.transpose(1, 0, 2).reshape(128, 16)),
                "wk": np.ascontiguousarray(wk.transpose(1, 0, 2).reshape(128, 16)),
                "wv": np.ascontiguousarray(wv.transpose(1, 2, 0).reshape(128, 16)),
                "wsub": np.ascontiguousarray(
                    W_out[:, 2 * c : 2 * c + 2, :].reshape(16, 128)
                ),
                "ident": ident,
            }
        )
    return in_maps


_CACHE = {}


def kernel(q, W_query, W_key, W_value, W_out, _trace=False, _trace_kwargs=None):
    nc = _CACHE.get("nc")
    if nc is None:
        nc = build_program()
        _CACHE["nc"] = nc
    in_maps = make_in_maps(q, W_query, W_key, W_value, W_out)
    res = run_bass_kernel_spmd(
        nc,
        in_maps,
        list(range(NCORES)),
        trace=_trace,
        **(_trace_kwargs or {}),
    )
    _CACHE["last_results"] = res
    parts = [np.asarray(res.results[i]["outp"], np.float64) for i in range(NCORES)]
    out = np.sum(parts, axis=0).astype(np.float32)
    return out.reshape(2, 256, 128)



# revision 13
# speedup vs baseline: 1.4618x; 1.4618x over previous
"""Trainium2 Bass kernel for nn_MultiHeadAttention_27693949124623.

Math (validated vs reference in fp64 numpy):
  The reference's "faithful split_heads" reinterprets the [H=8, B*N=512, 16]
  per-head projections as 16 independent full-attention instances
  (h2 in 0..8) x (b in 0..2), each over an effective sequence of 2048
  positions with head dim 2 and softmax over all 2048 keys:
     q index s in [0,2048):  g = 4*b + s//512 (original projection head),
                             t = s % 512     (row of qflat),
     per-instance Q/K/V components = column 2*h2+e of head g's projection.
  Output row r = b*256+n, col c = 16*j + 2*h2 + e gathers head outputs at
  s = 8*n+j, multiplied by W_out.reshape(128,128).

Sharding: core c handles h2 = c (head-parallel), both b instances.
Each core returns a partial [512, 128] (its h2 slice of W_out rows applied);
the host sums the 8 partials (the unshard step) and reshapes to (2,256,128).

Device pipeline per core (single packed-input DMA; q arrives pre-transposed):
  - one fp32r matmul projects Q^T/K^T for both instances: [32, 512]
    (row 16b+4gp+r: r=0,1 -> Qx,Qy; r=2,3 -> Kx,Ky for chunk gp)
  - V projections into [128, 4, 24] (cols 8e+g; 16..23 = ones for the
    softmax denominator)
  - main loop over (b, h, ci): scores [128 keys, 1024 q] = K2^T.T @ Q2
    on PE (K=2 contraction, raw fp32r), exp on ScalarE (PSUM -> SBUF),
    p^T @ [Vx, Vy, 1] accumulated on PE into a shared PSUM bank
  - gather accs[e, 8n+j] -> [16+16, 256] via 8 accumulating PE
    selection-matmuls with stride-8 rhs APs (no DMAs), DVE divide,
    final [16]x[128] matmuls against the core's W_out slice.
"""

import sys

for _p in ("/opt/trn_rl_repo",):
    if _p not in sys.path:
        sys.path.insert(0, _p)

from contextlib import ExitStack

import numpy as np

import concourse.bass as bass
import concourse.tile as tile
from concourse import mybir
from concourse.bass_utils import run_bass_kernel_spmd

F32 = mybir.dt.float32
F32R = mybir.dt.float32r
AF = mybir.ActivationFunctionType

NCORES = 8

# packed input layout (columns)
COL_QT = 0        # [128, 512] q^T
COL_WPROJ = 512   # [128, 32]  proj weights (0.5-scaled Wq/Wk cols)
COL_WV = 544      # [128, 16]  V weights (col 8e+g)
COL_WSUB = 560    # [16, 128]  W_out rows for this core
COL_SEL = 688     # [3, 384]   8 gather selection blocks [3, 48]
PACKED_W = 1072


def _emit(ctx: ExitStack, tc: tile.TileContext, io: dict):
    nc = tc.nc
    pin = io["packed"].ap()   # [128, 944] fp32r
    outp = io["outp"].ap()    # [512, 128] fp32

    const = ctx.enter_context(tc.tile_pool(name="const", bufs=1))
    sbp = ctx.enter_context(tc.tile_pool(name="sbp", bufs=3))
    sbm = ctx.enter_context(tc.tile_pool(name="sbm", bufs=1))
    # PSUM budget (8 banks of 2KB/partition):
    #   psc:  scores [128,1024] x2 bufs  = 4 banks
    #   pacc: acc16 [16,512] shared      = 1 bank
    #   pmisc: prep/gather/out tiles x2  = 2 banks
    psc = ctx.enter_context(tc.tile_pool(name="psc", bufs=2, space="PSUM"))
    pacc = ctx.enter_context(tc.tile_pool(name="pacc", bufs=1, space="PSUM"))
    pmisc = ctx.enter_context(tc.tile_pool(name="pmisc", bufs=2, space="PSUM"))

    # preload the exp table set first so the ~1.3us load overlaps the DMA
    dummy = const.tile([1, 8], F32, name="dummy")
    nc.vector.memset(dummy, 0.0)
    nc.scalar.activation(out=dummy, in_=dummy, func=AF.Exp)

    # ---- single packed constant load ------------------------------------
    packed = const.tile([128, PACKED_W], F32R, name="packed")
    nc.sync.dma_start(out=packed, in_=pin)
    qT = packed[:, COL_QT : COL_QT + 512]
    wproj = packed[:, COL_WPROJ : COL_WPROJ + 32]
    wv = packed[:, COL_WV : COL_WV + 16]
    wsub = packed[0:16, COL_WSUB : COL_WSUB + 128]
    sel = packed[0:3, COL_SEL : COL_SEL + 384]

    # ---- Q/K projections -> Qt/Kt [2, 2048] per instance -----------------
    # (matmul operands must sit at base partition 0, so Q/K live in 2-row
    # tiles; per-(b,gp) [2,512] projections are copied into the free dim,
    # spread across DVE / ScalarE / Pool)
    Qt = {b: const.tile([2, 2048], F32R, name=f"Qt{b}") for b in (0, 1)}
    Kt = {b: const.tile([2, 2048], F32R, name=f"Kt{b}") for b in (0, 1)}
    # NOTE: GPSIMD (Pool) cannot access PSUM, so PSUM->SBUF copies are
    # restricted to DVE and ScalarE.
    cp_engs = {
        0: [nc.vector, nc.scalar, nc.vector, nc.scalar, nc.vector,
            nc.scalar, nc.vector, nc.scalar],
        1: [nc.vector, nc.scalar, nc.vector, nc.scalar, nc.vector,
            nc.scalar, nc.vector, nc.scalar],
    }
    for b in (0, 1):
        for gp in range(4):
            col = COL_WPROJ + 16 * b + 4 * gp
            for k, dst in ((0, Qt[b]), (2, Kt[b])):
                pm = pmisc.tile([2, 512], F32, name=f"pj{b}{gp}{k}", tag="pm",
                                padded_shape=[128, 512])
                nc.tensor.matmul(
                    pm,
                    lhsT=packed[:, col + k : col + k + 2],
                    rhs=qT,
                    start=True,
                    stop=True,
                )
                eng = cp_engs[b][2 * gp + (k // 2)]
                dst_sl = dst[:, 512 * gp : 512 * (gp + 1)]
                if eng is nc.scalar:
                    nc.scalar.copy(out=dst_sl, in_=pm)
                else:
                    eng.tensor_copy(out=dst_sl, in_=pm)

    # ---- V projections -> gv [128, 4, 24]; cols 8e+g; 16..23 = ones ------
    gv = const.tile([128, 4, 24], F32R, name="gv")
    ones_sb = const.tile([128, 8], F32, name="ones_sb")
    nc.vector.memset(ones_sb, 1.0)
    for t4 in range(4):
        nc.vector.tensor_copy(out=gv[:, t4, 16:24], in_=ones_sb)
    for t4 in range(4):
        pg = pmisc.tile([128, 16], F32, name=f"pg{t4}", tag="pm",
                        padded_shape=[128, 512])
        nc.tensor.matmul(
            pg,
            lhsT=qT[:, 128 * t4 : 128 * (t4 + 1)],
            rhs=wv,
            start=True,
            stop=True,
        )
        nc.vector.tensor_copy(out=gv[:, t4, 0:16], in_=pg)
    gvv = gv.rearrange("p t (e g) -> p t g e", e=3)  # [128, 4, 8, 3]

    # two single-bank PSUM accumulators (matmul dst must be at partition 0)
    acc16 = {
        qi: pacc.tile([3, 512], F32, name=f"acc16_{qi}", tag=f"acc{qi}")
        for qi in (0, 1)
    }
    # per-instance accumulated [x, y, den] rows over all 2048 q positions
    accs = {
        b: sbm.tile([4, 2048], F32R, name=f"accs{b}", tag=f"accs{b}")
        for b in (0, 1)
    }
    o_sb = sbm.tile([128, 4, 128], F32, name="o_sb")

    steps = [(b, h, ci) for b in (0, 1) for h in (0, 1) for ci in range(16)]

    def emit_scores(i):
        b, h, ci = steps[i]
        g2, t4 = divmod(ci, 4)
        sps = psc.tile([128, 1024], F32, name=f"sps{b}{h}{ci}", tag="sps")
        for qi in range(2):
            qq = 2 * h + qi
            nc.tensor.matmul(
                sps[:, 512 * qi : 512 * (qi + 1)],
                lhsT=Kt[b][:, 128 * ci : 128 * ci + 128],
                rhs=Qt[b][:, 512 * qq : 512 * (qq + 1)],
                start=True,
                stop=True,
            )
        return sps

    def emit_tail(b):
        # gather accs[e, 8n+j] -> GP[2j+e (M) / 16+2j+e (den), n] on PE
        GP = pmisc.tile([48, 256], F32, name=f"GP{b}", tag="pm",
                        padded_shape=[128, 512])
        for j in range(8):
            nc.tensor.matmul(
                GP,
                lhsT=sel[:, 48 * j : 48 * j + 48],
                rhs=accs[b][0:3, j::8],
                start=(j == 0),
                stop=(j == 7),
            )
        rcp = sbm.tile([16, 256], F32R, name=f"rcp{b}", tag=f"rcp{b}")
        Md = sbm.tile([16, 256], F32R, name=f"Md{b}", tag=f"Md{b}")
        with nc.allow_low_precision(reason="f32r stores full fp32 bits"):
            nc.vector.reciprocal(out=rcp, in_=GP[32:48, :])
            nc.vector.tensor_mul(Md, GP[0:16, :], rcp)
        for ri in range(2):
            rr = 2 * b + ri
            pof = pmisc.tile([128, 128], F32, name=f"pof{rr}", tag="pm",
                             padded_shape=[128, 512])
            nc.tensor.matmul(
                pof, lhsT=Md[:, 128 * ri : 128 * (ri + 1)], rhs=wsub,
                start=True, stop=True,
            )
            eng = nc.scalar if ri == 0 else nc.vector
            if ri == 0:
                nc.scalar.copy(out=o_sb[:, rr, :], in_=pof)
            else:
                nc.vector.tensor_copy(out=o_sb[:, rr, :], in_=pof)
        nc.sync.dma_start(
            out=outp.rearrange("(r p) x -> p r x", p=128)[:, 2 * b : 2 * b + 2, :],
            in_=o_sb[:, 2 * b : 2 * b + 2, :],
        )

    # ---- main attention loop, software-skewed by one step ----------------
    sps_cur = emit_scores(0)
    for i, (b, h, ci) in enumerate(steps):
        g2, t4 = divmod(ci, 4)
        psb = sbp.tile([128, 1024], F32R, name=f"psb{b}{h}{ci}", tag="psb")
        nc.scalar.activation(out=psb, in_=sps_cur, func=AF.Exp)
        sps_nxt = emit_scores(i + 1) if i + 1 < len(steps) else None
        for qi in range(2):
            nc.tensor.matmul(
                acc16[qi],
                lhsT=gvv[:, t4, 4 * b + g2, :],
                rhs=psb[:, 512 * qi : 512 * (qi + 1)],
                start=(ci == 0),
                stop=(ci == 15),
            )
        sps_cur = sps_nxt
        if ci == 15:
            for qi in range(2):
                qq = 2 * h + qi
                nc.vector.tensor_copy(
                    out=accs[b][0:3, 512 * qq : 512 * (qq + 1)],
                    in_=acc16[qi],
                )
            if h == 1:
                emit_tail(b)


def _split_waits(nc):
    """Walrus codegen allows only ONE sync wait per engine instruction;
    Tile emits as many as the dependencies require. Split extras into
    preceding NoOps on the same engine (NX executes them in order)."""
    import bass_rust

    n = 0
    for fn in nc.m.functions:
        for blk in fn.blocks:
            newl = []
            for ins in blk.instructions:
                si = getattr(ins, "sync_info", None)
                waits = list(si.on_wait) if si is not None and si.on_wait else []
                if len(waits) > 1:
                    for k, w in enumerate(waits[:-1]):
                        nop = mybir.InstNoOp(name=f"{ins.name}-wsp{k}")
                        nop.engine = ins.engine
                        nop.sync_info = bass_rust.SyncInfo(on_wait=[w], on_update=[])
                        newl.append(nop)
                        n += 1
                    ins.sync_info = bass_rust.SyncInfo(
                        on_wait=[waits[-1]], on_update=list(si.on_update)
                    )
                newl.append(ins)
            try:
                blk.instructions = newl
            except Exception:
                blk.instructions.clear()
                blk.instructions.extend(newl)
    return n


def build_program():
    nc = bass.Bass("TRN2", target_bir_lowering=False, debug=False)
    io = {
        "packed": nc.declare_dram_parameter("packed", [128, PACKED_W], F32R,
                                            isOutput=False),
        "outp": nc.declare_dram_parameter("outp", [512, 128], F32, isOutput=True),
    }
    with tile.TileContext(nc) as tc:
        with ExitStack() as ctx:
            _emit(ctx, tc, io)
    _split_waits(nc)
    return nc


def make_in_maps(q, W_query, W_key, W_value, W_out):
    """Shard full inputs into per-core packed input maps (host-side, tiny)."""
    q = np.asarray(q, np.float32)
    W_query = np.asarray(W_query, np.float32)
    W_key = np.asarray(W_key, np.float32)
    W_value = np.asarray(W_value, np.float32)
    W_out = np.asarray(W_out, np.float32)
    qT = np.ascontiguousarray(q.reshape(512, 128).T)  # [128, 512]

    sel = np.zeros((3, 8, 48), np.float32)
    for j in range(8):
        sel[0, j, 2 * j] = 1.0
        sel[1, j, 2 * j + 1] = 1.0
        sel[2, j, 32 + 2 * j] = 1.0
        sel[2, j, 33 + 2 * j] = 1.0

    in_maps = []
    for c in range(NCORES):
        packed = np.zeros((128, PACKED_W), np.float32)
        packed[:, COL_QT : COL_QT + 512] = qT
        for b in (0, 1):
            for gp in range(4):
                g = 4 * b + gp
                col = COL_WPROJ + 16 * b + 4 * gp
                packed[:, col : col + 2] = 0.5 * W_query[g][:, 2 * c : 2 * c + 2]
                packed[:, col + 2 : col + 4] = 0.5 * W_key[g][:, 2 * c : 2 * c + 2]
        # V cols 8e+g
        packed[:, COL_WV : COL_WV + 16] = (
            W_value[:, :, 2 * c : 2 * c + 2].transpose(1, 2, 0).reshape(128, 16)
        )
        packed[0:16, COL_WSUB : COL_WSUB + 128] = W_out[:, 2 * c : 2 * c + 2, :].reshape(16, 128)
        packed[0:3, COL_SEL : COL_SEL + 384] = sel.reshape(3, 384)
        in_maps.append({"packed": packed})
    return in_maps


_CACHE = {}


def kernel(q, W_query, W_key, W_value, W_out, _trace=False, _trace_kwargs=None):
    nc = _CACHE.get("nc")
    if nc is None:
        nc = build_program()
        _CACHE["nc"] = nc
    in_maps = make_in_maps(q, W_query, W_key, W_value, W_out)
    res = run_bass_kernel_spmd(
        nc,
        in_maps,
        list(range(NCORES)),
        trace=_trace,
        **(_trace_kwargs or {}),
    )
    _CACHE["last_results"] = res
    parts = [np.asarray(res.results[i]["outp"], np.float64) for i in range(NCORES)]
    out = np.sum(parts, axis=0).astype(np.float32)
    return out.reshape(2, 256, 128)
